# revision 1
# baseline (speedup 1.0000x reference)
"""CSPN 3x3 propagation step on 8 Trainium2 NeuronCores.

out[b,0,r,c] = sum_k aff[b,k,r,c] * patch_k(cur)[r,c], with the center tap
(k=4) taken from coarse_seg instead of cur_seg. Zero padding at image edges.

Sharding: pure data parallel over batch (16 images -> 2 per core), one SPMD
Bass program run on all 8 cores with per-core input slices.

Per-core algorithm (per 512x512 image, packed as [128 partitions, 4 row
blocks, 512 cols]):
  - The tap row-shift (dy) is folded into the affinity DMA: plane k is
    loaded with a source row offset of -dy_k (A'_k[s] = aff_k[s-dy]).
    The overhanging first/last source row of the shifted window lands in
    an adjacent affinity plane (never out of bounds) and its product is
    provably never consumed.
  - The tap col-shift (dx) is a free-dim offset into a column-padded cur
    tile.
  - VectorEngine computes the 9 elementwise products P_k = A'_k * cur_x,
    then per-dy-group sums V_g (2 adds per group; optionally on GpSimd).
  - TensorEngine realigns the dy groups with shift-matrix matmuls
    (multiply by exact 0/1 -> bit-exact) accumulating in PSUM, including
    the cross-block boundary rows.
  - ScalarEngine evacuates PSUM -> SBUF; DMA stores the result.
"""

import sys

import numpy as np

if "/opt/trn_rl_repo" not in sys.path:
    sys.path.insert(0, "/opt/trn_rl_repo")

B_PER_CORE = 2
N_CORES = 8
H = 512
W = 512
NBLK = H // 128
WPAD = W + 2  # zero column on each side

_compiled = None
_compiled_reps = {}


def _shift_mats():
    """[128, 5, 128] f32: j=0 I, 1 Sd (k=m-1), 2 Su (k=m+1), 3 Ed, 4 Eu."""
    m = np.zeros((128, 5, 128), dtype=np.float32)
    for i in range(128):
        m[i, 0, i] = 1.0  # identity
    for i in range(127):
        m[i, 1, i + 1] = 1.0  # Sd: out[m] = in[m-1]
        m[i + 1, 2, i] = 1.0  # Su: out[m] = in[m+1]
    m[127, 3, 0] = 1.0  # Ed: out[0] = in[127]   (prev block)
    m[0, 4, 127] = 1.0  # Eu: out[127] = in[0]   (next block)
    return m


def _build_program(reps=1):
    """reps>1 unrolls the whole per-core computation `reps` times inside one
    NEFF — used only to measure kernel time through the dispatch noise."""
    import concourse.bacc as bacc
    import concourse.mybir as mybir
    import concourse.tile as tile

    fp32 = mybir.dt.float32

    nc = bacc.Bacc(
        "TRN2",
        target_bir_lowering=False,
        debug=False,
        enable_asserts=False,
        num_devices=N_CORES,
    )

    aff_d = nc.dram_tensor(
        "affinity", [B_PER_CORE, 9, H, W], fp32, kind="ExternalInput"
    ).ap()
    cur_d = nc.dram_tensor(
        "cur_seg", [B_PER_CORE, 1, H, W], fp32, kind="ExternalInput"
    ).ap()
    coa_d = nc.dram_tensor(
        "coarse_seg", [B_PER_CORE, 1, H, W], fp32, kind="ExternalInput"
    ).ap()
    smat_d = nc.dram_tensor("smats", [128, 5, 128], fp32, kind="ExternalInput").ap()
    out_d = nc.dram_tensor(
        "out", [B_PER_CORE, 1, H, W], fp32, kind="ExternalOutput"
    ).ap()

    with tile.TileContext(nc) as tc:
        with (
            tc.tile_pool(name="smat", bufs=1) as smat_pool,
            tc.tile_pool(name="aff", bufs=9) as aff_pool,
            tc.tile_pool(name="prod", bufs=7) as prod_pool,
            tc.tile_pool(name="cur", bufs=2) as cur_pool,
            tc.tile_pool(name="coa", bufs=2) as coa_pool,
            tc.tile_pool(name="acc", bufs=2) as acc_pool,
            tc.tile_pool(name="psum", bufs=8, space="PSUM") as psum_pool,
        ):
            tS = smat_pool.tile([128, 5, 128], fp32)
            SM_I, SM_SD, SM_SU, SM_ED, SM_EU = (tS[:, j, :] for j in range(5))
            smats_loaded = False

            for b in [bb for _ in range(reps) for bb in range(B_PER_CORE)]:
                last_img = b == B_PER_CORE - 1
                # --- cur tile [128, 4, 514], data in cols 1..512 ---
                # cur/coarse ride the ACT HWDGE ring; affinity mostly rides
                # the SP ring, so the two streams overlap.
                tM = cur_pool.tile([128, NBLK, WPAD], fp32, tag="cur")
                nc.vector.memset(tM[:, :, 0:1], 0.0)
                nc.vector.memset(tM[:, :, WPAD - 1 : WPAD], 0.0)
                cur_blocks = cur_d[b, 0].rearrange("(t p) c -> p t c", p=128)
                # split across both rings so cur completes ASAP (gates all
                # products)
                nc.scalar.dma_start(
                    out=tM[:, 0:2, 1 : W + 1], in_=cur_blocks[:, 0:2, :]
                )
                nc.sync.dma_start(
                    out=tM[:, 2:NBLK, 1 : W + 1], in_=cur_blocks[:, 2:NBLK, :]
                )

                # coarse is only needed by the center tap in the dy=0 group
                # (processed last) — defer its load past the dy=+1 planes
                tC = coa_pool.tile([128, NBLK, W], fp32, tag="coa")

                aff_flat = aff_d[b].flatten_outer_dims()  # [9*512, 512]

                acc = acc_pool.tile([128, NBLK, W], fp32, tag="acc")
                out_blocks = out_d[b, 0].rearrange("(t p) c -> p t c", p=128)
                psum_tiles = [
                    psum_pool.tile([128, W], fp32, tag="psum", name=f"ps{b}_{t}")
                    for t in range(NBLK)
                ]

                def _evac_store(t, out_ring):
                    nc.scalar.copy(out=acc[:, t, :], in_=psum_tiles[t])
                    out_ring.dma_start(out=out_blocks[:, t, :], in_=acc[:, t, :])

                def _load_group(g, add_eng, mul0_eng=None, act_dxi=1):
                    """Load the 3 planes of dy-group g (rows shifted -dy),
                    multiply against the shifted cur (or coarse for the
                    center tap), and tree-sum on add_eng. The first product
                    can run on a different engine (mul0_eng) to offload the
                    DVE. Returns V_g."""
                    dy = g - 1
                    Pg = []
                    for dxi in range(3):
                        k = 3 * g + dxi
                        dx = dxi - 1
                        ak = aff_pool.tile([128, NBLK, W], fp32, tag="aff")
                        start = 512 * k - dy
                        ring = nc.scalar if dxi == act_dxi else nc.sync
                        ring.dma_start(
                            out=ak[:],
                            in_=aff_flat[start : start + H, :].rearrange(
                                "(t p) c -> p t c", p=128
                            ),
                        )
                        pk = prod_pool.tile([128, NBLK, W], fp32, tag="prod")
                        src = tC[:] if k == 4 else tM[:, :, 1 + dx : 1 + dx + W]
                        meng = mul0_eng if (dxi == 0 and mul0_eng) else nc.vector
                        meng.tensor_mul(out=pk[:], in0=ak[:], in1=src)
                        Pg.append(pk)
                        if dxi == 1:
                            add_eng.tensor_add(out=Pg[0][:], in0=Pg[0][:], in1=Pg[1][:])
                    add_eng.tensor_add(out=Pg[0][:], in0=Pg[0][:], in1=Pg[2][:])
                    return Pg[0]

                # Groups are processed dy=+1, dy=-1, dy=0: each group's
                # shift-matmuls fire as soon as its sum exists, so by the
                # time the last group (dy=0, plain identity matmuls) lands,
                # the PE queue is nearly drained and the tail is short.
                # psum[t] accumulation order: Su(start), [Eu], Sd, [Ed],
                # I(stop).
                Vp1 = _load_group(2, nc.gpsimd, mul0_eng=nc.gpsimd)
                nc.scalar.dma_start(
                    out=tC[:], in_=coa_d[b, 0].rearrange("(t p) c -> p t c", p=128)
                )
                if not smats_loaded:
                    nc.scalar.dma_start(out=tS[:], in_=smat_d[:])
                    smats_loaded = True
                for t in range(NBLK):
                    nc.tensor.matmul(
                        psum_tiles[t], SM_SU, Vp1[:, t, :], start=True, stop=False
                    )
                    if t < NBLK - 1:
                        nc.tensor.matmul(
                            psum_tiles[t], SM_EU, Vp1[:, t + 1, :],
                            start=False, stop=False,
                        )

                Vm1 = _load_group(0, nc.gpsimd, mul0_eng=nc.gpsimd)
                for t in range(NBLK):
                    nc.tensor.matmul(
                        psum_tiles[t], SM_SD, Vm1[:, t, :], start=False, stop=False
                    )
                    if t > 0:
                        nc.tensor.matmul(
                            psum_tiles[t], SM_ED, Vm1[:, t - 1, :],
                            start=False, stop=False,
                        )

                if not last_img:
                    # --- dy = 0 group, whole-plane path ---
                    V0 = _load_group(1, nc.vector, act_dxi=2)
                    for t in range(NBLK):
                        nc.tensor.matmul(
                            psum_tiles[t], SM_I, V0[:, t, :], start=False, stop=True
                        )
                        _evac_store(t, nc.scalar)
                else:
                    # --- dy = 0 group for the last image: block-halves.
                    # Half 0 (blocks 0-1) loads via ACT while half 1
                    # (blocks 2-3) loads via SP concurrently; psum[0]/[1]
                    # complete as soon as half 0's sum exists, so their
                    # evacuation and stores overlap half 1's compute. The
                    # final serial chain is half-sized.
                    for h in range(2):
                        ring = nc.scalar if h == 0 else nc.sync
                        Ph = []
                        for dxi in range(3):
                            k = 3 + dxi
                            dx = dxi - 1
                            ak = aff_pool.tile([128, 2, W], fp32, tag="aff")
                            start = 512 * k + 256 * h
                            ring.dma_start(
                                out=ak[:],
                                in_=aff_flat[start : start + 256, :].rearrange(
                                    "(t p) c -> p t c", p=128
                                ),
                            )
                            pk = prod_pool.tile([128, 2, W], fp32, tag="prod")
                            src = (
                                tC[:, 2 * h : 2 * h + 2, :]
                                if k == 4
                                else tM[:, 2 * h : 2 * h + 2, 1 + dx : 1 + dx + W]
                            )
                            nc.vector.tensor_mul(out=pk[:], in0=ak[:], in1=src)
                            Ph.append(pk)
                            if dxi == 1:
                                nc.vector.tensor_add(
                                    out=Ph[0][:], in0=Ph[0][:], in1=Ph[1][:]
                                )
                        nc.vector.tensor_add(out=Ph[0][:], in0=Ph[0][:], in1=Ph[2][:])
                        for th in range(2):
                            t = 2 * h + th
                            nc.tensor.matmul(
                                psum_tiles[t], SM_I, Ph[0][:, th, :],
                                start=False, stop=True,
                            )
                            _evac_store(t, nc.scalar if th == 0 else nc.sync)

    nc.compile()
    return nc


def _get_program(reps=1):
    global _compiled
    if reps != 1:
        if reps not in _compiled_reps:
            _compiled_reps[reps] = _build_program(reps)
        return _compiled_reps[reps]
    if _compiled is None:
        _compiled = _build_program()
    return _compiled


def _in_maps(affinity, cur_seg, coarse_seg):
    smats = _shift_mats()
    maps = []
    for j in range(N_CORES):
        s = slice(j * B_PER_CORE, (j + 1) * B_PER_CORE)
        maps.append(
            {
                "affinity": np.ascontiguousarray(affinity[s]),
                "cur_seg": np.ascontiguousarray(cur_seg[s]),
                "coarse_seg": np.ascontiguousarray(coarse_seg[s]),
                "smats": smats,
            }
        )
    return maps


def kernel(affinity, cur_seg, coarse_seg, i=None, **_unused):
    from concourse.bass_utils import run_bass_kernel_spmd

    nc = _get_program()

    affinity = np.ascontiguousarray(affinity, dtype=np.float32)
    cur_seg = np.ascontiguousarray(cur_seg, dtype=np.float32)
    coarse_seg = np.ascontiguousarray(coarse_seg, dtype=np.float32)

    res = run_bass_kernel_spmd(
        nc, _in_maps(affinity, cur_seg, coarse_seg), core_ids=list(range(N_CORES))
    )
    out = np.concatenate([r["out"] for r in res.results], axis=0)
    return out



# revision 5
# speedup vs baseline: 1.0702x; 1.0702x over previous
"""CSPN 3x3 propagation step on 8 Trainium2 NeuronCores.

out[b,0,r,c] = sum_k aff[b,k,r,c] * patch_k(cur)[r,c], with the center tap
(k=4) taken from coarse_seg instead of cur_seg. Zero padding at image edges.

Sharding: pure data parallel over batch (16 images -> 2 per core), one SPMD
Bass program run on all 8 cores with per-core input slices.

Per-core algorithm (per 512x512 image, packed as [128 partitions, 4 row
blocks, 512+2 cols]):
  - The tap row-shift (dy) is folded into three separate DMA loads of
    cur_seg (dy=-1, 0, +1): tile slot s holds cur[s+dy]. The shifted
    windows are expressed as two in-bounds DMAs (a 127-partition bulk
    window plus a 1-partition block-boundary edge) and the out-of-range
    image-boundary row is memset to zero. No partition realignment (no
    TensorEngine, no PSUM) is ever needed.
  - The tap col-shift (dx) is a free-dim offset into the column-padded
    cur tiles.
  - Affinity planes are loaded unshifted, one 3MB DMA per dy-group of 3
    planes ("(k t p) c -> p (k t) c").
  - Compute is pure elementwise: DVE takes taps 0-3,5 (5 muls + 4 adds),
    GpSimd takes taps 4,6-8 (4 muls + 3 adds), DVE does the final add
    and the result DMAs straight out of SBUF.
  - Ring split: affinity rides the SP HWDGE ring; cur/coarse loads and
    output stores ride the ACT ring. Loads for both images are issued
    before any store so neither ring head-of-line blocks.
"""

import sys

import numpy as np

if "/opt/trn_rl_repo" not in sys.path:
    sys.path.insert(0, "/opt/trn_rl_repo")

B_PER_CORE = 2
N_CORES = 8
H = 512
W = 512
NBLK = H // 128  # 4 row blocks of 128 partitions
WPAD = W + 2  # zero column on each side

_compiled = None
_compiled_reps = {}


def _build_program(reps=1):
    """reps>1 repeats the whole per-core computation `reps` times inside one
    NEFF — used only to measure kernel time through the dispatch noise."""
    import concourse.bacc as bacc
    import concourse.mybir as mybir
    import concourse.tile as tile

    fp32 = mybir.dt.float32

    nc = bacc.Bacc(
        "TRN2",
        target_bir_lowering=False,
        debug=False,
        enable_asserts=False,
        num_devices=N_CORES,
    )

    aff_d = nc.dram_tensor(
        "affinity", [B_PER_CORE, 9, H, W], fp32, kind="ExternalInput"
    ).ap()
    cur_d = nc.dram_tensor(
        "cur_seg", [B_PER_CORE, 1, H, W], fp32, kind="ExternalInput"
    ).ap()
    coa_d = nc.dram_tensor(
        "coarse_seg", [B_PER_CORE, 1, H, W], fp32, kind="ExternalInput"
    ).ap()
    out_d = nc.dram_tensor(
        "out", [B_PER_CORE, 1, H, W], fp32, kind="ExternalOutput"
    ).ap()

    with tile.TileContext(nc) as tc:
        with (
            tc.tile_pool(name="aff", bufs=2) as aff_pool,
            tc.tile_pool(name="cur", bufs=2) as cur_pool,
            tc.tile_pool(name="coa", bufs=2) as coa_pool,
            tc.tile_pool(name="dacc", bufs=2) as dacc_pool,
            tc.tile_pool(name="gacc", bufs=2) as gacc_pool,
            tc.tile_pool(name="dtmp", bufs=1) as dtmp_pool,
            tc.tile_pool(name="gtmp", bufs=1) as gtmp_pool,
        ):
            for _rep in range(reps):
                # ---- phase 1: issue all loads for both images ----
                curs = []  # per image: (t0, tM, tP, tCoa)
                affs = []  # per image: [g0, g1, g2] tiles [128, 12, 512]
                for b in range(B_PER_CORE):
                    V = cur_d[b, 0].rearrange("(t p) c -> p t c", p=128)
                    C = coa_d[b, 0].rearrange("(t p) c -> p t c", p=128)

                    # dy=0: straight copy into cols 1..512
                    t0 = cur_pool.tile([128, NBLK, WPAD], fp32, tag="cur0")
                    nc.vector.memset(t0[:, :, 0:1], 0.0)
                    nc.vector.memset(t0[:, :, WPAD - 1 : WPAD], 0.0)
                    nc.scalar.dma_start(out=t0[:, :, 1 : W + 1], in_=V)

                    # dy=-1: slot s holds cur[s-1]; slot 0 is zero
                    tM = cur_pool.tile([128, NBLK, WPAD], fp32, tag="curM")
                    nc.vector.memset(tM[:, :, 0:1], 0.0)
                    nc.vector.memset(tM[:, :, WPAD - 1 : WPAD], 0.0)
                    # zero whole block 0 first; the bulk DMA then overwrites
                    # partitions 1..127, leaving slot 0 zero (BIR forbids
                    # single-partition memsets at nonzero partition starts)
                    nc.vector.memset(tM[:, 0:1, :], 0.0)
                    nc.scalar.dma_start(
                        out=tM[1:128, :, 1 : W + 1], in_=V[0:127, :, :]
                    )
                    nc.scalar.dma_start(
                        out=tM[0:1, 1:NBLK, 1 : W + 1], in_=V[127:128, 0 : NBLK - 1, :]
                    )

                    # dy=+1: slot s holds cur[s+1]; slot 511 is zero
                    tP = cur_pool.tile([128, NBLK, WPAD], fp32, tag="curP")
                    nc.gpsimd.memset(tP[:, :, 0:1], 0.0)
                    nc.gpsimd.memset(tP[:, :, WPAD - 1 : WPAD], 0.0)
                    nc.gpsimd.memset(tP[:, NBLK - 1 : NBLK, :], 0.0)
                    nc.scalar.dma_start(
                        out=tP[0:127, :, 1 : W + 1], in_=V[1:128, :, :]
                    )
                    nc.scalar.dma_start(
                        out=tP[127:128, 0 : NBLK - 1, 1 : W + 1], in_=V[0:1, 1:NBLK, :]
                    )

                    # center tap source (no shift, no padding needed)
                    tCoa = coa_pool.tile([128, NBLK, W], fp32, tag="coa")
                    nc.scalar.dma_start(out=tCoa[:], in_=C)

                    curs.append((t0, tM, tP, tCoa))

                    gtiles = []
                    for g in range(3):
                        ag = aff_pool.tile([128, 3 * NBLK, W], fp32, tag="aff")
                        nc.sync.dma_start(
                            out=ag[:],
                            in_=aff_d[b, 3 * g : 3 * g + 3]
                            .flatten_outer_dims()
                            .rearrange("(k t p) c -> p (k t) c", k=3, p=128),
                        )
                        gtiles.append(ag)
                    affs.append(gtiles)

                # ---- phase 2: compute + store per image ----
                for b in range(B_PER_CORE):
                    t0c, tM, tP, tCoa = curs[b]
                    g0, g1, g2 = affs[b]

                    def aplane(gt, k):
                        kk = k % 3
                        return gt[:, NBLK * kk : NBLK * kk + NBLK, :]

                    def src(k):
                        dx = k % 3 - 1
                        if k < 3:
                            return tM[:, :, 1 + dx : 1 + dx + W]
                        if k == 4:
                            return tCoa[:]
                        if k in (3, 5):
                            return t0c[:, :, 1 + dx : 1 + dx + W]
                        return tP[:, :, 1 + dx : 1 + dx + W]

                    # DVE: taps 0,1,2,3,5 -> da
                    da = dacc_pool.tile([128, NBLK, W], fp32, tag="dacc")
                    dt = dtmp_pool.tile([128, NBLK, W], fp32, tag="dtmp")
                    nc.vector.tensor_mul(out=da[:], in0=aplane(g0, 0), in1=src(0))
                    nc.vector.tensor_mul(out=dt[:], in0=aplane(g0, 1), in1=src(1))
                    nc.vector.tensor_add(out=da[:], in0=da[:], in1=dt[:])
                    nc.vector.tensor_mul(out=dt[:], in0=aplane(g0, 2), in1=src(2))
                    nc.vector.tensor_add(out=da[:], in0=da[:], in1=dt[:])
                    nc.vector.tensor_mul(out=dt[:], in0=aplane(g1, 3), in1=src(3))
                    nc.vector.tensor_add(out=da[:], in0=da[:], in1=dt[:])
                    nc.vector.tensor_mul(out=dt[:], in0=aplane(g1, 5), in1=src(5))
                    nc.vector.tensor_add(out=da[:], in0=da[:], in1=dt[:])

                    # GpSimd: taps 6,7,8,4 -> ga
                    ga = gacc_pool.tile([128, NBLK, W], fp32, tag="gacc")
                    gt = gtmp_pool.tile([128, NBLK, W], fp32, tag="gtmp")
                    nc.gpsimd.tensor_mul(out=ga[:], in0=aplane(g2, 6), in1=src(6))
                    nc.gpsimd.tensor_mul(out=gt[:], in0=aplane(g2, 7), in1=src(7))
                    nc.gpsimd.tensor_add(out=ga[:], in0=ga[:], in1=gt[:])
                    nc.gpsimd.tensor_mul(out=gt[:], in0=aplane(g2, 8), in1=src(8))
                    nc.gpsimd.tensor_add(out=ga[:], in0=ga[:], in1=gt[:])
                    nc.gpsimd.tensor_mul(out=gt[:], in0=aplane(g1, 4), in1=src(4))
                    nc.gpsimd.tensor_add(out=ga[:], in0=ga[:], in1=gt[:])

                    nc.vector.tensor_add(out=da[:], in0=da[:], in1=ga[:])

                    out_blocks = out_d[b, 0].rearrange("(t p) c -> p t c", p=128)
                    nc.scalar.dma_start(out=out_blocks[:], in_=da[:])

    nc.compile()
    return nc


def _get_program(reps=1):
    global _compiled
    if reps != 1:
        if reps not in _compiled_reps:
            _compiled_reps[reps] = _build_program(reps)
        return _compiled_reps[reps]
    if _compiled is None:
        _compiled = _build_program()
    return _compiled


def _in_maps(affinity, cur_seg, coarse_seg):
    maps = []
    for j in range(N_CORES):
        s = slice(j * B_PER_CORE, (j + 1) * B_PER_CORE)
        maps.append(
            {
                "affinity": np.ascontiguousarray(affinity[s]),
                "cur_seg": np.ascontiguousarray(cur_seg[s]),
                "coarse_seg": np.ascontiguousarray(coarse_seg[s]),
            }
        )
    return maps


def kernel(affinity, cur_seg, coarse_seg, i=None, **_unused):
    from concourse.bass_utils import run_bass_kernel_spmd

    nc = _get_program()

    affinity = np.ascontiguousarray(affinity, dtype=np.float32)
    cur_seg = np.ascontiguousarray(cur_seg, dtype=np.float32)
    coarse_seg = np.ascontiguousarray(coarse_seg, dtype=np.float32)

    res = run_bass_kernel_spmd(
        nc, _in_maps(affinity, cur_seg, coarse_seg), core_ids=list(range(N_CORES))
    )
    out = np.concatenate([r["out"] for r in res.results], axis=0)
    return out


# revision 6
# speedup vs baseline: 30.3276x; 28.3373x over previous
"""CSPN 3x3 propagation step on 8 Trainium2 NeuronCores.

out[b,0,r,c] = sum_k aff[b,k,r,c] * patch_k(cur)[r,c], with the center tap
(k=4) taken from coarse_seg instead of cur_seg. Zero padding at image edges.

Sharding: pure data parallel over batch (16 images -> 2 per core), one SPMD
Bass program run on all 8 cores with per-core input slices.

Per-core algorithm (per 512x512 image, packed as [128 partitions, 4 row
blocks, 512 cols]):
  - The tap row-shift (dy) is folded into the affinity DMA: plane k is
    loaded with a source row offset of -dy_k (A'_k[s] = aff_k[s-dy]).
    The overhanging first/last source row of the shifted window lands in
    an adjacent affinity plane (never out of bounds) and its product is
    provably never consumed.
  - The tap col-shift (dx) is a free-dim offset into a column-padded cur
    tile.
  - VectorEngine computes the 9 elementwise products P_k = A'_k * cur_x,
    then per-dy-group sums V_g (2 adds per group; optionally on GpSimd).
  - TensorEngine realigns the dy groups with shift-matrix matmuls
    (multiply by exact 0/1 -> bit-exact) accumulating in PSUM, including
    the cross-block boundary rows.
  - ScalarEngine evacuates PSUM -> SBUF; DMA stores the result.
"""

import sys

import numpy as np

if "/opt/trn_rl_repo" not in sys.path:
    sys.path.insert(0, "/opt/trn_rl_repo")

B_PER_CORE = 2
N_CORES = 8
H = 512
W = 512
NBLK = H // 128
WPAD = W + 2  # zero column on each side

_compiled = None
_compiled_reps = {}


def _shift_mats():
    """[128, 5, 128] f32: j=0 I, 1 Sd (k=m-1), 2 Su (k=m+1), 3 Ed, 4 Eu."""
    m = np.zeros((128, 5, 128), dtype=np.float32)
    for i in range(128):
        m[i, 0, i] = 1.0  # identity
    for i in range(127):
        m[i, 1, i + 1] = 1.0  # Sd: out[m] = in[m-1]
        m[i + 1, 2, i] = 1.0  # Su: out[m] = in[m+1]
    m[127, 3, 0] = 1.0  # Ed: out[0] = in[127]   (prev block)
    m[0, 4, 127] = 1.0  # Eu: out[127] = in[0]   (next block)
    return m


def _build_program(reps=1):
    """reps>1 unrolls the whole per-core computation `reps` times inside one
    NEFF — used only to measure kernel time through the dispatch noise."""
    import concourse.bacc as bacc
    import concourse.mybir as mybir
    import concourse.tile as tile

    fp32 = mybir.dt.float32

    nc = bacc.Bacc(
        "TRN2",
        target_bir_lowering=False,
        debug=False,
        enable_asserts=False,
        num_devices=N_CORES,
    )

    aff_d = nc.dram_tensor(
        "affinity", [B_PER_CORE, 9, H, W], fp32, kind="ExternalInput"
    ).ap()
    cur_d = nc.dram_tensor(
        "cur_seg", [B_PER_CORE, 1, H, W], fp32, kind="ExternalInput"
    ).ap()
    coa_d = nc.dram_tensor(
        "coarse_seg", [B_PER_CORE, 1, H, W], fp32, kind="ExternalInput"
    ).ap()
    smat_d = nc.dram_tensor("smats", [128, 5, 128], fp32, kind="ExternalInput").ap()
    out_d = nc.dram_tensor(
        "out", [B_PER_CORE, 1, H, W], fp32, kind="ExternalOutput"
    ).ap()

    with tile.TileContext(nc) as tc:
        with (
            tc.tile_pool(name="smat", bufs=1) as smat_pool,
            tc.tile_pool(name="aff", bufs=9) as aff_pool,
            tc.tile_pool(name="prod", bufs=7) as prod_pool,
            tc.tile_pool(name="cur", bufs=2) as cur_pool,
            tc.tile_pool(name="coa", bufs=2) as coa_pool,
            tc.tile_pool(name="acc", bufs=2) as acc_pool,
            tc.tile_pool(name="psum", bufs=8, space="PSUM") as psum_pool,
        ):
            tS = smat_pool.tile([128, 5, 128], fp32)
            SM_I, SM_SD, SM_SU, SM_ED, SM_EU = (tS[:, j, :] for j in range(5))
            smats_loaded = False

            for b in [bb for _ in range(reps) for bb in range(B_PER_CORE)]:
                last_img = b == B_PER_CORE - 1
                # --- cur tile [128, 4, 514], data in cols 1..512 ---
                # cur/coarse ride the ACT HWDGE ring; affinity mostly rides
                # the SP ring, so the two streams overlap.
                tM = cur_pool.tile([128, NBLK, WPAD], fp32, tag="cur")
                nc.vector.memset(tM[:, :, 0:1], 0.0)
                nc.vector.memset(tM[:, :, WPAD - 1 : WPAD], 0.0)
                cur_blocks = cur_d[b, 0].rearrange("(t p) c -> p t c", p=128)
                # split across both rings so cur completes ASAP (gates all
                # products)
                nc.scalar.dma_start(
                    out=tM[:, 0:2, 1 : W + 1], in_=cur_blocks[:, 0:2, :]
                )
                nc.sync.dma_start(
                    out=tM[:, 2:NBLK, 1 : W + 1], in_=cur_blocks[:, 2:NBLK, :]
                )

                # coarse is only needed by the center tap in the dy=0 group
                # (processed last) — defer its load past the dy=+1 planes
                tC = coa_pool.tile([128, NBLK, W], fp32, tag="coa")

                aff_flat = aff_d[b].flatten_outer_dims()  # [9*512, 512]

                acc = acc_pool.tile([128, NBLK, W], fp32, tag="acc")
                out_blocks = out_d[b, 0].rearrange("(t p) c -> p t c", p=128)
                psum_tiles = [
                    psum_pool.tile([128, W], fp32, tag="psum", name=f"ps{b}_{t}")
                    for t in range(NBLK)
                ]

                def _evac_store(t, out_ring):
                    nc.scalar.copy(out=acc[:, t, :], in_=psum_tiles[t])
                    out_ring.dma_start(out=out_blocks[:, t, :], in_=acc[:, t, :])

                def _load_group(g, add_eng, mul0_eng=None, act_dxi=1):
                    """Load the 3 planes of dy-group g (rows shifted -dy),
                    multiply against the shifted cur (or coarse for the
                    center tap), and tree-sum on add_eng. The first product
                    can run on a different engine (mul0_eng) to offload the
                    DVE. Returns V_g."""
                    dy = g - 1
                    Pg = []
                    for dxi in range(3):
                        k = 3 * g + dxi
                        dx = dxi - 1
                        ak = aff_pool.tile([128, NBLK, W], fp32, tag="aff")
                        start = 512 * k - dy
                        ring = nc.scalar if dxi == act_dxi else nc.sync
                        ring.dma_start(
                            out=ak[:],
                            in_=aff_flat[start : start + H, :].rearrange(
                                "(t p) c -> p t c", p=128
                            ),
                        )
                        pk = prod_pool.tile([128, NBLK, W], fp32, tag="prod")
                        src = tC[:] if k == 4 else tM[:, :, 1 + dx : 1 + dx + W]
                        meng = mul0_eng if (dxi == 0 and mul0_eng) else nc.vector
                        meng.tensor_mul(out=pk[:], in0=ak[:], in1=src)
                        Pg.append(pk)
                        if dxi == 1:
                            add_eng.tensor_add(out=Pg[0][:], in0=Pg[0][:], in1=Pg[1][:])
                    add_eng.tensor_add(out=Pg[0][:], in0=Pg[0][:], in1=Pg[2][:])
                    return Pg[0]

                # Groups are processed dy=+1, dy=-1, dy=0: each group's
                # shift-matmuls fire as soon as its sum exists, so by the
                # time the last group (dy=0, plain identity matmuls) lands,
                # the PE queue is nearly drained and the tail is short.
                # psum[t] accumulation order: Su(start), [Eu], Sd, [Ed],
                # I(stop).
                Vp1 = _load_group(2, nc.gpsimd, mul0_eng=nc.gpsimd)
                nc.scalar.dma_start(
                    out=tC[:], in_=coa_d[b, 0].rearrange("(t p) c -> p t c", p=128)
                )
                if not smats_loaded:
                    nc.scalar.dma_start(out=tS[:], in_=smat_d[:])
                    smats_loaded = True
                for t in range(NBLK):
                    nc.tensor.matmul(
                        psum_tiles[t], SM_SU, Vp1[:, t, :], start=True, stop=False
                    )
                    if t < NBLK - 1:
                        nc.tensor.matmul(
                            psum_tiles[t], SM_EU, Vp1[:, t + 1, :],
                            start=False, stop=False,
                        )

                Vm1 = _load_group(0, nc.gpsimd, mul0_eng=nc.gpsimd)
                for t in range(NBLK):
                    nc.tensor.matmul(
                        psum_tiles[t], SM_SD, Vm1[:, t, :], start=False, stop=False
                    )
                    if t > 0:
                        nc.tensor.matmul(
                            psum_tiles[t], SM_ED, Vm1[:, t - 1, :],
                            start=False, stop=False,
                        )

                if not last_img:
                    # --- dy = 0 group, whole-plane path ---
                    V0 = _load_group(1, nc.vector, act_dxi=2)
                    for t in range(NBLK):
                        nc.tensor.matmul(
                            psum_tiles[t], SM_I, V0[:, t, :], start=False, stop=True
                        )
                        _evac_store(t, nc.scalar)
                else:
                    # --- dy = 0 group for the last image: block-halves.
                    # Half 0 (blocks 0-1) loads via ACT while half 1
                    # (blocks 2-3) loads via SP concurrently; psum[0]/[1]
                    # complete as soon as half 0's sum exists, so their
                    # evacuation and stores overlap half 1's compute. The
                    # final serial chain is half-sized.
                    for h in range(2):
                        ring = nc.scalar if h == 0 else nc.sync
                        Ph = []
                        for dxi in range(3):
                            k = 3 + dxi
                            dx = dxi - 1
                            ak = aff_pool.tile([128, 2, W], fp32, tag="aff")
                            start = 512 * k + 256 * h
                            ring.dma_start(
                                out=ak[:],
                                in_=aff_flat[start : start + 256, :].rearrange(
                                    "(t p) c -> p t c", p=128
                                ),
                            )
                            pk = prod_pool.tile([128, 2, W], fp32, tag="prod")
                            src = (
                                tC[:, 2 * h : 2 * h + 2, :]
                                if k == 4
                                else tM[:, 2 * h : 2 * h + 2, 1 + dx : 1 + dx + W]
                            )
                            nc.vector.tensor_mul(out=pk[:], in0=ak[:], in1=src)
                            Ph.append(pk)
                            if dxi == 1:
                                nc.vector.tensor_add(
                                    out=Ph[0][:], in0=Ph[0][:], in1=Ph[1][:]
                                )
                        nc.vector.tensor_add(out=Ph[0][:], in0=Ph[0][:], in1=Ph[2][:])
                        for th in range(2):
                            t = 2 * h + th
                            nc.tensor.matmul(
                                psum_tiles[t], SM_I, Ph[0][:, th, :],
                                start=False, stop=True,
                            )
                            _evac_store(t, nc.scalar if th == 0 else nc.sync)

    nc.compile()
    return nc


def _get_program(reps=1):
    global _compiled
    if reps != 1:
        if reps not in _compiled_reps:
            _compiled_reps[reps] = _build_program(reps)
        return _compiled_reps[reps]
    if _compiled is None:
        _compiled = _build_program()
    return _compiled


def _in_maps(affinity, cur_seg, coarse_seg):
    smats = _shift_mats()
    maps = []
    for j in range(N_CORES):
        s = slice(j * B_PER_CORE, (j + 1) * B_PER_CORE)
        maps.append(
            {
                "affinity": np.ascontiguousarray(affinity[s]),
                "cur_seg": np.ascontiguousarray(cur_seg[s]),
                "coarse_seg": np.ascontiguousarray(coarse_seg[s]),
                "smats": smats,
            }
        )
    return maps


def kernel(affinity, cur_seg, coarse_seg, i=None, **_unused):
    from concourse.bass_utils import run_bass_kernel_spmd

    nc = _get_program()

    affinity = np.ascontiguousarray(affinity, dtype=np.float32)
    cur_seg = np.ascontiguousarray(cur_seg, dtype=np.float32)
    coarse_seg = np.ascontiguousarray(coarse_seg, dtype=np.float32)

    res = run_bass_kernel_spmd(
        nc, _in_maps(affinity, cur_seg, coarse_seg), core_ids=list(range(N_CORES))
    )
    out = np.concatenate([r["out"] for r in res.results], axis=0)
    return out



# revision 22
# speedup vs baseline: 90.7875x; 2.9936x over previous
"""CSPN 3x3 propagation step on 8 Trainium2 NeuronCores.

out[b,0,r,c] = sum_k aff[b,k,r,c] * patch_k(cur)[r,c], with the center tap
(k=4) taken from coarse_seg instead of cur_seg. Zero padding at image edges.

Sharding: pure data parallel over batch (16 images -> 2 per core), one SPMD
Bass program run on all 8 cores with per-core input slices.

Per-core algorithm (per 512x512 image, packed as [128 partitions, 4 row
blocks, 512 cols]):
  - The tap row-shift (dy) is folded into the affinity DMA: plane k is
    loaded with a source row offset of -dy_k (A'_k[s] = aff_k[s-dy]).
    The overhanging first/last source row of the shifted window lands in
    an adjacent affinity plane (never out of bounds) and its product is
    provably never consumed.
  - The tap col-shift (dx) is a free-dim offset into a column-padded cur
    tile.
  - 6 of 9 affinity planes load via SWDGE cast-DMA fp32->bf16 (halves
    their SBUF-port-side bytes, the ~435 GB/s binding constraint); the
    middle plane of each group stays fp32 on the HWDGE rings so both DGE
    paths run in parallel (all-SWDGE serializes on the single Q7 queue).
  - DVE computes only the 9 elementwise products P_k = A'_k * cur_x,
    written directly as bf16 (free cast on the multiply output). There are
    NO tree-sum adds: every product feeds the TensorEngine shift-matrix
    matmuls (bf16, 4x fp32 PE rate) and PSUM performs the entire 9-tap
    accumulation in fp32. This removes the engine add chain that was the
    measured critical path.
  - ScalarEngine evacuates PSUM -> SBUF; DMA stores the fp32 result.
    Rel err ~2.2e-3 from bf16 rounding (gate: 2e-2).
"""

import sys

import numpy as np

if "/opt/trn_rl_repo" not in sys.path:
    sys.path.insert(0, "/opt/trn_rl_repo")

B_PER_CORE = 2
N_CORES = 8
H = 512
W = 512
NBLK = H // 128
WPAD = W + 2  # zero column on each side

_compiled = None
_compiled_reps = {}


def _shift_mats():
    """[128, 5, 128] f32: j=0 I, 1 Sd (k=m-1), 2 Su (k=m+1), 3 Ed, 4 Eu."""
    m = np.zeros((128, 5, 128), dtype=np.float32)
    for i in range(128):
        m[i, 0, i] = 1.0  # identity
    for i in range(127):
        m[i, 1, i + 1] = 1.0  # Sd: out[m] = in[m-1]
        m[i + 1, 2, i] = 1.0  # Su: out[m] = in[m+1]
    m[127, 3, 0] = 1.0  # Ed: out[0] = in[127]   (prev block)
    m[0, 4, 127] = 1.0  # Eu: out[127] = in[0]   (next block)
    return m


def _build_program(reps=1):
    """reps>1 unrolls the whole per-core computation `reps` times inside one
    NEFF — used only to measure kernel time through the dispatch noise."""
    import concourse.bacc as bacc
    import concourse.mybir as mybir
    import concourse.tile as tile

    fp32 = mybir.dt.float32
    bf16 = mybir.dt.bfloat16

    nc = bacc.Bacc(
        "TRN2",
        target_bir_lowering=False,
        debug=False,
        enable_asserts=False,
        num_devices=N_CORES,
    )

    aff_d = nc.dram_tensor(
        "affinity", [B_PER_CORE, 9, H, W], fp32, kind="ExternalInput"
    ).ap()
    cur_d = nc.dram_tensor(
        "cur_seg", [B_PER_CORE, 1, H, W], fp32, kind="ExternalInput"
    ).ap()
    coa_d = nc.dram_tensor(
        "coarse_seg", [B_PER_CORE, 1, H, W], fp32, kind="ExternalInput"
    ).ap()
    smat_d = nc.dram_tensor("smats", [128, 5, 128], fp32, kind="ExternalInput").ap()
    out_d = nc.dram_tensor(
        "out", [B_PER_CORE, 1, H, W], fp32, kind="ExternalOutput"
    ).ap()

    with tile.TileContext(nc) as tc:
        with (
            tc.tile_pool(name="smat", bufs=1) as smat_pool,
            tc.tile_pool(name="aff", bufs=9) as aff_pool,
            tc.tile_pool(name="prod", bufs=7) as prod_pool,
            tc.tile_pool(name="cur", bufs=2) as cur_pool,
            tc.tile_pool(name="coa", bufs=2) as coa_pool,
            tc.tile_pool(name="acc", bufs=2) as acc_pool,
            tc.tile_pool(name="psum", bufs=8, space="PSUM") as psum_pool,
        ):
            tS = smat_pool.tile([128, 5, 128], fp32)
            tSb = smat_pool.tile([128, 5, 128], bf16, tag="smatb")
            SM_I, SM_SD, SM_SU, SM_ED, SM_EU = (tSb[:, j, :] for j in range(5))
            smats_loaded = False

            for b in [bb for _ in range(reps) for bb in range(B_PER_CORE)]:
                last_img = b == B_PER_CORE - 1
                # --- cur tile [128, 4, 514], data in cols 1..512 ---
                # cur/coarse ride the ACT HWDGE ring; affinity mostly rides
                # the SP ring, so the two streams overlap.
                tM = cur_pool.tile([128, NBLK, WPAD], fp32, tag="cur")
                nc.vector.memset(tM[:, :, 0:1], 0.0)
                nc.vector.memset(tM[:, :, WPAD - 1 : WPAD], 0.0)
                cur_blocks = cur_d[b, 0].rearrange("(t p) c -> p t c", p=128)
                # split across both rings so cur completes ASAP (gates all
                # products)
                nc.scalar.dma_start(
                    out=tM[:, 0:2, 1 : W + 1], in_=cur_blocks[:, 0:2, :]
                )
                nc.sync.dma_start(
                    out=tM[:, 2:NBLK, 1 : W + 1], in_=cur_blocks[:, 2:NBLK, :]
                )

                # coarse is only needed by the center tap in the dy=0 group
                # (processed last) — defer its load past the dy=+1 planes
                tC = coa_pool.tile([128, NBLK, W], fp32, tag="coa")

                aff_flat = aff_d[b].flatten_outer_dims()  # [9*512, 512]

                acc = acc_pool.tile([128, NBLK, W], fp32, tag="acc")
                out_blocks = out_d[b, 0].rearrange("(t p) c -> p t c", p=128)
                psum_tiles = [
                    psum_pool.tile([128, W], fp32, tag="psum", name=f"ps{b}_{t}")
                    for t in range(NBLK)
                ]

                def _evac_store(t, out_ring):
                    nc.scalar.copy(out=acc[:, t, :], in_=psum_tiles[t])
                    out_ring.dma_start(out=out_blocks[:, t, :], in_=acc[:, t, :])

                def _load_group(g, add_eng, mul0_eng=None, act_dxi=1):
                    """Load the 3 planes of dy-group g (rows shifted -dy)
                    and multiply against the shifted cur (or coarse for the
                    center tap) into bf16 products. No tree-sum: every
                    product feeds the PSUM accumulation directly (the PE at
                    bf16 rate absorbs 3x the matmuls, freeing DVE/GpSimd of
                    all adds). Returns [P_a, P_b, P_c]."""
                    dy = g - 1
                    Pg = []
                    for dxi in range(3):
                        k = 3 * g + dxi
                        dx = dxi - 1
                        start = 512 * k - dy
                        if dxi == 1:
                            # middle plane of each group: HWDGE fp32 (keeps
                            # the two HW rings busy, offloads the SWDGE Q7)
                            ak = aff_pool.tile([128, NBLK, W], fp32, tag="affh")
                            ring = nc.scalar if g == 1 else nc.sync
                            ring.dma_start(
                                out=ak[:],
                                in_=aff_flat[start : start + H, :].rearrange(
                                    "(t p) c -> p t c", p=128
                                ),
                            )
                        else:
                            ak = aff_pool.tile([128, NBLK, W], bf16, tag="aff")
                            nc.gpsimd.dma_start(
                                out=ak[:],
                                in_=aff_flat[start : start + H, :].rearrange(
                                    "(t p) c -> p t c", p=128
                                ),
                            )
                        pk = prod_pool.tile([128, NBLK, W], bf16, tag="prod")
                        src = tC[:] if k == 4 else tM[:, :, 1 + dx : 1 + dx + W]
                        meng = mul0_eng if (dxi == 0 and mul0_eng) else nc.vector
                        meng.tensor_mul(out=pk[:], in0=ak[:], in1=src)
                        Pg.append(pk)
                    return Pg

                # Groups are processed dy=+1, dy=-1, dy=0: each group's
                # shift-matmuls fire as soon as its sum exists, so by the
                # time the last group (dy=0, plain identity matmuls) lands,
                # the PE queue is nearly drained and the tail is short.
                # psum[t] accumulation order: Su(start), [Eu], Sd, [Ed],
                # I(stop).
                Pp1 = _load_group(2, nc.gpsimd, mul0_eng=nc.gpsimd)
                nc.scalar.dma_start(
                    out=tC[:], in_=coa_d[b, 0].rearrange("(t p) c -> p t c", p=128)
                )
                if not smats_loaded:
                    nc.scalar.dma_start(out=tS[:], in_=smat_d[:])
                    nc.scalar.copy(out=tSb[:], in_=tS[:])
                    smats_loaded = True
                for t in range(NBLK):
                    for pi, pk in enumerate(Pp1):
                        nc.tensor.matmul(
                            psum_tiles[t], SM_SU, pk[:, t, :],
                            start=(pi == 0), stop=False,
                        )
                        if t < NBLK - 1:
                            nc.tensor.matmul(
                                psum_tiles[t], SM_EU, pk[:, t + 1, :],
                                start=False, stop=False,
                            )

                Pm1 = _load_group(0, nc.gpsimd, mul0_eng=nc.gpsimd)
                for t in range(NBLK):
                    for pk in Pm1:
                        nc.tensor.matmul(
                            psum_tiles[t], SM_SD, pk[:, t, :], start=False, stop=False
                        )
                        if t > 0:
                            nc.tensor.matmul(
                                psum_tiles[t], SM_ED, pk[:, t - 1, :],
                                start=False, stop=False,
                            )

                if not last_img:
                    # --- dy = 0 group, whole-plane path ---
                    P0g = _load_group(1, nc.vector, act_dxi=2)
                    for t in range(NBLK):
                        for pi, pk in enumerate(P0g):
                            nc.tensor.matmul(
                                psum_tiles[t], SM_I, pk[:, t, :],
                                start=False, stop=(pi == 2),
                            )
                        _evac_store(t, nc.scalar)
                else:
                    # --- dy = 0 group for the last image: block-halves.
                    # Half 0 (blocks 0-1) loads via ACT while half 1
                    # (blocks 2-3) loads via SP concurrently; psum[0]/[1]
                    # complete as soon as half 0's sum exists, so their
                    # evacuation and stores overlap half 1's compute. The
                    # final serial chain is half-sized.
                    for h in range(2):
                        ring = nc.scalar if h == 0 else nc.sync
                        Ph = []
                        for dxi in range(3):
                            k = 3 + dxi
                            dx = dxi - 1
                            ak = aff_pool.tile([128, 2, W], bf16, tag="aff")
                            start = 512 * k + 256 * h
                            nc.gpsimd.dma_start(
                                out=ak[:],
                                in_=aff_flat[start : start + 256, :].rearrange(
                                    "(t p) c -> p t c", p=128
                                ),
                            )
                            pk = prod_pool.tile([128, 2, W], bf16, tag="prod")
                            src = (
                                tC[:, 2 * h : 2 * h + 2, :]
                                if k == 4
                                else tM[:, 2 * h : 2 * h + 2, 1 + dx : 1 + dx + W]
                            )
                            nc.vector.tensor_mul(out=pk[:], in0=ak[:], in1=src)
                            Ph.append(pk)
                        for th in range(2):
                            t = 2 * h + th
                            for pi, pk in enumerate(Ph):
                                nc.tensor.matmul(
                                    psum_tiles[t], SM_I, pk[:, th, :],
                                    start=False, stop=(pi == 2),
                                )
                            _evac_store(t, nc.scalar if th == 0 else nc.sync)

    nc.compile()
    return nc


def _get_program(reps=1):
    global _compiled
    if reps != 1:
        if reps not in _compiled_reps:
            _compiled_reps[reps] = _build_program(reps)
        return _compiled_reps[reps]
    if _compiled is None:
        _compiled = _build_program()
    return _compiled


def _in_maps(affinity, cur_seg, coarse_seg):
    smats = _shift_mats()
    maps = []
    for j in range(N_CORES):
        s = slice(j * B_PER_CORE, (j + 1) * B_PER_CORE)
        maps.append(
            {
                "affinity": np.ascontiguousarray(affinity[s]),
                "cur_seg": np.ascontiguousarray(cur_seg[s]),
                "coarse_seg": np.ascontiguousarray(coarse_seg[s]),
                "smats": smats,
            }
        )
    return maps


def kernel(affinity, cur_seg, coarse_seg, i=None, **_unused):
    from concourse.bass_utils import run_bass_kernel_spmd

    nc = _get_program()

    affinity = np.ascontiguousarray(affinity, dtype=np.float32)
    cur_seg = np.ascontiguousarray(cur_seg, dtype=np.float32)
    coarse_seg = np.ascontiguousarray(coarse_seg, dtype=np.float32)

    res = run_bass_kernel_spmd(
        nc, _in_maps(affinity, cur_seg, coarse_seg), core_ids=list(range(N_CORES))
    )
    out = np.concatenate([r["out"] for r in res.results], axis=0)
    return out

